# revision 1
# baseline (speedup 1.0000x reference)
"""Trainium2 Bass kernel for nn_CorefModel (LSTM + span pooling + mention MLP +
windowed pairwise precedent MLP + softmax).

Sharding: data-parallel over batch B=8 across the 8 NeuronCores (one batch row
per core, all parameters replicated). No collectives.

Per-core pipeline (all layouts transposed so the partition dim is 128):
  A) indirect-DMA embedding gather -> fp16 -> DRAM -> transposing DMA -> we^T
  B) X^T[1024,512] = Wih^T @ we^T (fp16 matmul, fp32 psum, bias folded in)
  C) 512-step LSTM recurrence: gates g^T[128,8] via 16 small matmuls/step with
     Whh (fp16, gate-permuted to g,i,f,o order) stationary; X-add + nonlinearity
     fused into per-column ScalarE activation ops (per-partition bias operand).
  D) span pooling: PE transpose seq^T -> seq, span sums as matmul against a
     host-built 0/1 indicator, PE transpose back -> tgt^T.
  E) mention MLP (fp32, transposed so biases are per-partition scalars).
  F) pairwise MLP (fp16): feat^T built with sliding-window / broadcast access
     patterns (precedent window j = i-50+k is just a shifted slice), 2-layer
     MLP in N=512 blocks, scalar head via K-partition-reduction matmuls.
  G) scores + masked softmax. softmax shift-invariance removes the ms_i
     broadcast: cols 0:50 = ms_j + ps + mask, epsilon col = -ms_i.
"""
import numpy as np

B, W, M, P = 8, 512, 128, 50
V, E, L, H = 50000, 300, 256, 512
G = 4 * L
NCORES = 8
NEG_INF = -1.0e30

_CACHE = {}


# ---------------------------------------------------------------- host prep --
def _perm_gifo():
    # reference gate order (i,f,g,o); device uses (g,i,f,o)
    return np.concatenate([np.arange(512, 768), np.arange(0, 256),
                           np.arange(256, 512), np.arange(768, 1024)])


def _blocked(w, kchunks, hchunks):
    """[K,HH] -> [128, kchunks*hchunks*128] with col block (k*hchunks+h)*128."""
    K, HH = w.shape
    out = np.zeros((128, kchunks * hchunks * 128), w.dtype)
    for k in range(kchunks):
        kp = min(128, K - k * 128)
        for h in range(hchunks):
            blk = w[k * 128:k * 128 + kp, h * 128:(h + 1) * 128]
            out[:kp, (k * hchunks + h) * 128:(k * hchunks + h + 1) * 128] = blk
    return out


def _chunk_cols(v, n):
    """[n*128] -> [128, n] (col j = chunk j)."""
    return np.ascontiguousarray(v.reshape(n, 128).T)


def _prep_shared(inputs):
    f32, f16 = np.float32, np.float16
    perm = _perm_gifo()
    Wih = np.asarray(inputs["Wih"], f32)[:, perm]
    Whh = np.asarray(inputs["Whh"], f32)[:, perm]
    bias = (np.asarray(inputs["bih"], f32) + np.asarray(inputs["bhh"], f32))[perm]

    wih_pad = np.zeros((304, G), f16)
    wih_pad[:E] = Wih.astype(f16)

    i_idx = np.arange(M)[:, None]
    k_idx = np.arange(P)[None, :]
    valid = k_idx < np.minimum(i_idx, P)
    maskinf = np.where(valid, 0.0, NEG_INF).astype(f32)

    return {
        "emb": np.asarray(inputs["emb"], f32),
        "wih16": wih_pad,
        "whh16": Whh.astype(f16),
        "biasg": _chunk_cols(bias, 8).astype(f32),
        "wm1": _blocked(np.asarray(inputs["Wm1"], f32), 2, 4),
        "wm2": _blocked(np.asarray(inputs["Wm2"], f32), 4, 4),
        "bm": np.concatenate([_chunk_cols(np.asarray(inputs["bm1"], f32), 4),
                              _chunk_cols(np.asarray(inputs["bm2"], f32), 4)], 1),
        "wmv": _chunk_cols(np.asarray(inputs["wm"], f32), 4),
        "wa1": _blocked(np.asarray(inputs["Wa1"], np.float32).astype(f16), 6, 4),
        "wa2": _blocked(np.asarray(inputs["Wa2"], np.float32).astype(f16), 4, 4),
        "ba": np.concatenate([_chunk_cols(np.asarray(inputs["ba1"], f32), 4),
                              _chunk_cols(np.asarray(inputs["ba2"], f32), 4)], 1),
        "wav": _chunk_cols(np.asarray(inputs["wa"], np.float32), 4).astype(f16),
        "maskinf": maskinf,
        "ident": np.eye(128, dtype=f32),
    }


def _prep_core(inputs, b):
    f32 = np.float32
    word = np.asarray(inputs["word_seq"][b], np.int32)
    starts = np.asarray(inputs["span_starts"][b], np.int64)
    lens = np.asarray(inputs["span_lengths"][b], np.int64)
    ends = np.clip(starts + lens, 0, W)
    t_idx = np.arange(W)[:, None]
    ind_full = ((t_idx >= starts[None, :]) & (t_idx < ends[None, :])).astype(f32)
    # ind[p, q*128+m] = ind_full[q*128+p, m]
    ind = np.ascontiguousarray(
        ind_full.reshape(4, 128, M).transpose(1, 0, 2).reshape(128, 4 * M))
    widx = np.ascontiguousarray(word.reshape(4, 128).T).astype(np.int32)
    return {"widx": widx, "ind": ind}


# ------------------------------------------------------------ program build --
def _build_program():
    import concourse.bacc as bacc
    import concourse.tile as tile
    from concourse import mybir
    import concourse.bass as bass

    f32, f16, i32 = mybir.dt.float32, mybir.dt.float16, mybir.dt.int32
    AF = mybir.ActivationFunctionType
    OP = mybir.AluOpType

    nc = bacc.Bacc("TRN2", target_bir_lowering=False, debug=False)

    def din(name, shape, dt):
        return nc.dram_tensor(name, shape, dt, kind="ExternalInput").ap()

    emb_d = din("emb", [V, E], f32)
    widx_d = din("widx", [128, 4], i32)
    wih_d = din("wih16", [304, G], f16)
    whh_d = din("whh16", [L, G], f16)
    biasg_d = din("biasg", [128, 8], f32)
    ind_d = din("ind", [128, 4 * M], f32)
    wm1_d = din("wm1", [128, 2 * 4 * 128], f32)
    wm2_d = din("wm2", [128, 4 * 4 * 128], f32)
    bm_d = din("bm", [128, 8], f32)
    wmv_d = din("wmv", [128, 4], f32)
    wa1_d = din("wa1", [128, 6 * 4 * 128], f16)
    wa2_d = din("wa2", [128, 4 * 4 * 128], f16)
    ba_d = din("ba", [128, 8], f32)
    wav_d = din("wav", [128, 4], f16)
    mask_d = din("maskinf", [128, P], f32)
    ident_d = din("ident", [128, 128], f32)

    we16_d = nc.dram_tensor("we16s", [W, 384], f16).ap()
    ms_d = nc.dram_tensor("mss", [M, 1], f32).ap()
    ps_d = nc.dram_tensor("pss", [1, M * P], f32).ap()
    out_d = nc.dram_tensor("o", [M, P + 1], f32, kind="ExternalOutput").ap()

    def ap3(base, off_elems, dims):
        """Manual AP on the same tensor: dims = [[stride, num], ...] (free),
        partition dim copied from base."""
        return bass.AP(tensor=base.tensor, offset=base.offset + off_elems,
                       ap=[base.ap[0]] + dims)

    with tile.TileContext(nc) as tc:
        from contextlib import ExitStack
        ctx = ExitStack()
        with ctx:
            singles = ctx.enter_context(tc.tile_pool(name="singles", bufs=1))
            bigctx = ExitStack()
            bigs = bigctx.enter_context(tc.tile_pool(name="bigs", bufs=1))

            # ---- LSTM-phase tensors (freed before pairwise phase) ------------
            weT = bigs.tile([128, 3, W], f16)       # we^T k-chunks
            wih_sb = bigs.tile([128, 3, 8, 128], f16)
            whh_sb = bigs.tile([128, 2, 8, 128], f16)
            biasg_sb = bigs.tile([128, 8], f32)
            XT = bigs.tile([128, W, 8], f32)        # X^T: (t, gate-chunk)
            seqT = bigs.tile([128, 2, W], f32)      # h^T history
            ident_sb = bigs.tile([128, 128], f32)
            ind_sb = bigs.tile([128, 4, M], f32)
            c32 = bigs.tile([128, 2], f32)
            h16 = bigs.tile([128, 2], f16)

            # ---- persistent SBUF tensors -------------------------------------
            wm1_sb = singles.tile([128, 2, 4, 128], f32)
            wm2_sb = singles.tile([128, 4, 4, 128], f32)
            bm_sb = singles.tile([128, 8], f32)
            wmv_sb = singles.tile([128, 4], f32)
            wa1_sb = singles.tile([128, 6, 4, 128], f16)
            wa2_sb = singles.tile([128, 4, 4, 128], f16)
            ba_sb = singles.tile([128, 8], f32)
            wav_sb = singles.tile([128, 4], f16)
            mask_sb = singles.tile([128, P], f32)
            tgtT32 = singles.tile([128, 2, M], f32)
            tgtT16 = singles.tile([128, 2, M], f16)
            m1T = singles.tile([128, 4, M], f32)
            m2T = singles.tile([128, 4, M], f32)
            ms_sb = singles.tile([1, M], f32)
            msi_sb = singles.tile([128, 1], f32)
            msj_sb = singles.tile([128, P], f32)
            psM_sb = singles.tile([128, P], f32)
            idx_sb = singles.tile([128, 4], i32)

            # weight / static DMAs (no deps -> scheduled early)
            nc.sync.dma_start(out=idx_sb[:], in_=widx_d[:])
            for k in range(3):
                kp = 128 if k < 2 else 48
                nc.sync.dma_start(out=wih_sb[0:kp, k, :, :],
                                  in_=wih_d[k * 128:k * 128 + kp, :])
            for k in range(2):
                nc.sync.dma_start(out=whh_sb[:, k, :, :],
                                  in_=whh_d[k * 128:(k + 1) * 128, :])
            nc.sync.dma_start(out=biasg_sb[:], in_=biasg_d[:])
            nc.sync.dma_start(out=ident_sb[:], in_=ident_d[:])
            nc.sync.dma_start(out=ind_sb[:], in_=ind_d[:])
            nc.sync.dma_start(out=wm1_sb[:], in_=wm1_d[:])
            nc.sync.dma_start(out=wm2_sb[:], in_=wm2_d[:])
            nc.sync.dma_start(out=bm_sb[:], in_=bm_d[:])
            nc.sync.dma_start(out=wmv_sb[:], in_=wmv_d[:])
            nc.sync.dma_start(out=wa1_sb[:], in_=wa1_d[:])
            nc.sync.dma_start(out=wa2_sb[:], in_=wa2_d[:])
            nc.sync.dma_start(out=ba_sb[:], in_=ba_d[:])
            nc.sync.dma_start(out=wav_sb[:], in_=wav_d[:])
            nc.sync.dma_start(out=mask_sb[:], in_=mask_d[:])

            # ---- phase A: embedding gather + transpose -----------------------
            with tc.tile_pool(name="gath", bufs=2) as gpool:
                for g in range(4):
                    wet = gpool.tile([128, 384], f32, tag="wet")
                    nc.vector.memset(wet[:, E:384], 0.0)
                    nc.gpsimd.indirect_dma_start(
                        out=wet[:, 0:E], out_offset=None, in_=emb_d[:],
                        in_offset=bass.IndirectOffsetOnAxis(
                            ap=idx_sb[:, g:g + 1], axis=0))
                    # cast f32 -> f16 during DMA (SWDGE)
                    nc.gpsimd.dma_start(out=we16_d[g * 128:(g + 1) * 128, :],
                                        in_=wet[:])
                for c in range(3):
                    nc.sync.dma_start(out=weT[:, c, :],
                                      in_=we16_d[:, c * 128:(c + 1) * 128], transpose=True)

            # ---- phase B: X^T = Wih^T @ we^T + bias --------------------------
            with tc.tile_pool(name="xps", bufs=2, space="PSUM") as xps:
                for j in range(8):
                    px = xps.tile([128, W], f32, tag="px")
                    for k, kp in enumerate([128, 128, 48]):
                        nc.tensor.matmul(out=px[:], lhsT=wih_sb[0:kp, k, j, :],
                                         rhs=weT[0:kp, k, :],
                                         start=(k == 0), stop=(k == 2))
                    nc.scalar.activation(out=XT[:, :, j], in_=px[:],
                                         func=AF.Identity,
                                         bias=biasg_sb[:, j:j + 1])

            # ---- phase C: LSTM recurrence ------------------------------------
            with tc.tile_pool(name="lps", bufs=2, space="PSUM") as lps, \
                 tc.tile_pool(name="lsb", bufs=3) as lsb:
                nc.vector.memset(c32[:], 0.0)
                nc.vector.memset(h16[:], 0.0)
                h_prev = h16
                for t in range(W):
                    pg = [lps.tile([128, 2], f32, tag=f"pg{p}", name=f"pg{p}_{t}") for p in range(4)]
                    for j in range(8):
                        for k in range(2):
                            nc.tensor.matmul(out=pg[j // 2][:, j % 2:j % 2 + 1],
                                             lhsT=whh_sb[:, k, j, :],
                                             rhs=h_prev[:, k:k + 1],
                                             start=(k == 0), stop=(k == 1))
                    ga = lsb.tile([128, 8], f32, tag="ga")
                    for j in range(8):
                        nc.scalar.activation(
                            out=ga[:, j:j + 1], in_=pg[j // 2][:, j % 2:j % 2 + 1],
                            func=(AF.Tanh if j < 2 else AF.Sigmoid),
                            bias=XT[:, t, j:j + 1])
                    ig = lsb.tile([128, 2], f32, tag="ig")
                    nc.vector.tensor_tensor(out=ig[:], in0=ga[:, 2:4],
                                            in1=ga[:, 0:2], op=OP.mult)
                    fc = lsb.tile([128, 2], f32, tag="fc")
                    nc.vector.tensor_tensor(out=fc[:], in0=ga[:, 4:6],
                                            in1=c32[:], op=OP.mult)
                    nc.vector.tensor_tensor(out=c32[:], in0=ig[:], in1=fc[:],
                                            op=OP.add)
                    tch = lsb.tile([128, 2], f32, tag="tch")
                    nc.scalar.activation(out=tch[:], in_=c32[:], func=AF.Tanh)
                    hn = lsb.tile([128, 2], f16, tag="hn")
                    nc.vector.tensor_tensor(out=hn[:], in0=ga[:, 6:8],
                                            in1=tch[:], op=OP.mult)
                    nc.vector.tensor_tensor(out=seqT[:, :, t], in0=ga[:, 6:8],
                                            in1=tch[:], op=OP.mult)
                    h_prev = hn

            # ---- phase D: span pooling ---------------------------------------
            with tc.tile_pool(name="dps", bufs=4, space="PSUM") as dps, \
                 tc.tile_pool(name="dsb", bufs=2) as dsb:
                tgt_ps = dps.tile([128, 2 * 128], f32, tag="tgt")
                for q in range(4):
                    seq_q = dsb.tile([128, 2, 128], f32, tag="seqq")
                    for c in range(2):
                        pt = dps.tile([128, 128], f32, tag="pt")
                        nc.tensor.transpose(out=pt[:],
                                            in_=seqT[:, c, q * 128:(q + 1) * 128],
                                            identity=ident_sb[:])
                        nc.vector.tensor_copy(out=seq_q[:, c, :], in_=pt[:])
                    nc.tensor.matmul(out=tgt_ps[:], lhsT=ind_sb[:, q, :],
                                     rhs=seq_q[:].rearrange("p c t -> p (c t)"),
                                     start=(q == 0), stop=(q == 3))
                tgt_sb = dsb.tile([128, 256], f32, tag="tgtsb")
                nc.vector.tensor_copy(out=tgt_sb[:], in_=tgt_ps[:])
                for c in range(2):
                    pt2 = dps.tile([128, 128], f32, tag="pt")
                    nc.tensor.transpose(out=pt2[:],
                                        in_=tgt_sb[:, c * 128:(c + 1) * 128],
                                        identity=ident_sb[:])
                    nc.vector.tensor_copy(out=tgtT32[:, c, :], in_=pt2[:])
                    nc.vector.tensor_copy(out=tgtT16[:, c, :], in_=pt2[:])

            bigctx.close()  # free LSTM-phase SBUF before pairwise

            # ---- phase E: mention MLP + ms -----------------------------------
            with tc.tile_pool(name="eps", bufs=2, space="PSUM") as eps:
                for h in range(4):
                    pm = eps.tile([128, M], f32, tag="pm")
                    for k in range(2):
                        nc.tensor.matmul(out=pm[:], lhsT=wm1_sb[:, k, h, :],
                                         rhs=tgtT32[:, k, :],
                                         start=(k == 0), stop=(k == 1))
                    nc.scalar.activation(out=m1T[:, h, :], in_=pm[:],
                                         func=AF.Relu, bias=bm_sb[:, h:h + 1])
                for h in range(4):
                    pm = eps.tile([128, M], f32, tag="pm")
                    for k in range(4):
                        nc.tensor.matmul(out=pm[:], lhsT=wm2_sb[:, k, h, :],
                                         rhs=m1T[:, k, :],
                                         start=(k == 0), stop=(k == 3))
                    nc.scalar.activation(out=m2T[:, h, :], in_=pm[:],
                                         func=AF.Relu, bias=bm_sb[:, 4 + h:5 + h])
                pms = eps.tile([1, M], f32, tag="pms")
                for k in range(4):
                    nc.tensor.matmul(out=pms[:], lhsT=wmv_sb[:, k:k + 1],
                                     rhs=m2T[:, k, :],
                                     start=(k == 0), stop=(k == 3))
                nc.vector.tensor_copy(out=ms_sb[:], in_=pms[:])
                nc.sync.dma_start(out=ms_d[:], in_=ms_sb[:])
                # ms_i per-partition
                nc.sync.dma_start(out=msi_sb[:], in_=ms_d[:])
                # ms_j sliding window: i>=50 -> ms[i-50+k]; i<50 -> ms[k]
                nc.sync.dma_start(
                    out=msj_sb[P:M, :],
                    in_=bass.AP(tensor=ms_d.tensor, offset=0,
                                ap=[[1, M - P], [1, P]]))
                nc.sync.dma_start(
                    out=msj_sb[0:P, :],
                    in_=bass.AP(tensor=ms_d.tensor, offset=0,
                                ap=[[0, P], [1, P]]))

            # ---- phase F: pairwise MLP ---------------------------------------
            NPAIR = M * P  # 6400
            NA = P * P     # 2500 (region i<50)
            blocks = [(s, min(512, NPAIR - s)) for s in range(0, NPAIR, 512)]
            with tc.tile_pool(name="h1p", bufs=1) as h1p, \
                 tc.tile_pool(name="fps", bufs=2, space="PSUM") as fps, \
                 tc.tile_pool(name="fpssb", bufs=3) as fps_sb:
                h1T = h1p.tile([128, 4, NPAIR], f16)
                featp = ExitStack()
                featpool = featp.enter_context(tc.tile_pool(name="feat", bufs=1))
                featT = featpool.tile([128, 6, NPAIR], f16)
                for c in range(2):
                    base = tgtT16[:, c, :]       # [128, 128] fp16
                    # jvec (chunk c): A: tgt[k]; B: tgt[i-50+k]
                    nc.vector.tensor_copy(
                        out=featT[:, c, 0:NA].rearrange("p (i k) -> p i k", k=P),
                        in_=ap3(base, 0, [[0, P], [1, P]]))
                    nc.vector.tensor_copy(
                        out=featT[:, c, NA:NPAIR].rearrange("p (i k) -> p i k", k=P),
                        in_=ap3(base, 0, [[1, M - P], [1, P]]))
                    # ivec (chunk 2+c): A: tgt[i]; B: tgt[i]
                    nc.vector.tensor_copy(
                        out=featT[:, 2 + c, 0:NA].rearrange("p (i k) -> p i k", k=P),
                        in_=ap3(base, 0, [[1, P], [0, P]]))
                    nc.vector.tensor_copy(
                        out=featT[:, 2 + c, NA:NPAIR].rearrange("p (i k) -> p i k", k=P),
                        in_=ap3(base, P, [[1, M - P], [0, P]]))
                    # prod (chunk 4+c)
                    nc.vector.tensor_tensor(out=featT[:, 4 + c, :],
                                            in0=featT[:, c, :],
                                            in1=featT[:, 2 + c, :], op=OP.mult)
                for n0, nb in blocks:
                    for h in range(4):
                        p1 = fps.tile([128, 512], f32, tag="p1")
                        for k in range(6):
                            nc.tensor.matmul(out=p1[:, 0:nb],
                                             lhsT=wa1_sb[:, k, h, :],
                                             rhs=featT[:, k, n0:n0 + nb],
                                             start=(k == 0), stop=(k == 5))
                        nc.scalar.activation(out=h1T[:, h, n0:n0 + nb],
                                             in_=p1[:, 0:nb], func=AF.Relu,
                                             bias=ba_sb[:, h:h + 1])
                featp.close()  # free featT before h2T allocates
                with tc.tile_pool(name="h2p", bufs=1) as h2p:
                    h2T = h2p.tile([128, 4, NPAIR], f16)
                    for n0, nb in blocks:
                        for h in range(4):
                            p2 = fps.tile([128, 512], f32, tag="p1")
                            for k in range(4):
                                nc.tensor.matmul(out=p2[:, 0:nb],
                                                 lhsT=wa2_sb[:, k, h, :],
                                                 rhs=h1T[:, k, n0:n0 + nb],
                                                 start=(k == 0), stop=(k == 3))
                            nc.scalar.activation(out=h2T[:, h, n0:n0 + nb],
                                                 in_=p2[:, 0:nb], func=AF.Relu,
                                                 bias=ba_sb[:, 4 + h:5 + h])
                    for n0, nb in blocks:
                        pps = fps.tile([1, 512], f32, tag="pps")
                        for k in range(4):
                            nc.tensor.matmul(out=pps[:, 0:nb],
                                             lhsT=wav_sb[:, k:k + 1],
                                             rhs=h2T[:, k, n0:n0 + nb],
                                             start=(k == 0), stop=(k == 3))
                        pse = fps_sb.tile([1, 512], f32, tag="pse",
                                          name=f"pse_{n0}")
                        nc.vector.tensor_copy(out=pse[:, 0:nb],
                                              in_=pps[:, 0:nb])
                        nc.sync.dma_start(out=ps_d[:, n0:n0 + nb],
                                          in_=pse[:, 0:nb])
                nc.sync.dma_start(
                    out=psM_sb[:],
                    in_=bass.AP(tensor=ps_d.tensor, offset=0,
                                ap=[[P, M], [1, P]]))

            # ---- phase G: scores + softmax -----------------------------------
            with tc.tile_pool(name="gsb", bufs=1) as gsb:
                sc = gsb.tile([128, P + 1], f32)
                nc.vector.tensor_tensor(out=sc[:, 0:P], in0=psM_sb[:],
                                        in1=msj_sb[:], op=OP.add)
                nc.vector.tensor_tensor(out=sc[:, 0:P], in0=sc[:, 0:P],
                                        in1=mask_sb[:], op=OP.add)
                nc.vector.tensor_scalar_mul(sc[:, P:P + 1], msi_sb[:], -1.0)
                mx = gsb.tile([128, 1], f32)
                nc.vector.tensor_reduce(out=mx[:], in_=sc[:],
                                        axis=mybir.AxisListType.X,
                                        op=OP.max, negate=True)
                ex = gsb.tile([128, P + 1], f32)
                sm = gsb.tile([128, 1], f32)
                nc.scalar.activation(out=ex[:], in_=sc[:], func=AF.Exp,
                                     bias=mx[:], accum_out=sm[:])
                rs = gsb.tile([128, 1], f32)
                nc.vector.reciprocal(out=rs[:], in_=sm[:])
                ot = gsb.tile([128, P + 1], f32)
                nc.vector.tensor_scalar_mul(ot[:], ex[:], rs[:])
                nc.sync.dma_start(out=out_d[:], in_=ot[:])

    nc.compile()
    return nc


# -------------------------------------------------------------------- entry --
def kernel(**inputs):
    import os
    from concourse.bass_utils import run_bass_kernel_spmd

    if "nc" not in _CACHE:
        _CACHE["nc"] = _build_program()
    nc = _CACHE["nc"]

    shared = _prep_shared(inputs)
    in_maps = []
    for b in range(NCORES):
        m = dict(shared)
        m.update(_prep_core(inputs, b))
        in_maps.append(m)

    trace = bool(os.environ.get("COREF_TRACE"))
    res = run_bass_kernel_spmd(nc, in_maps, core_ids=list(range(NCORES)),
                               trace=trace)
    kernel.last_exec_ns = res.exec_time_ns
    kernel.last_results = res
    out = np.stack([res.results[i]["o"] for i in range(NCORES)])
    return out.astype(np.float32)


if __name__ == "__main__":
    import jax
    jax.config.update("jax_platforms", "cpu")
    import reference as ref
    inputs = ref.setup_inputs()
    expected = np.asarray(jax.device_get(ref.reference(**inputs)))
    got = kernel(**{k: np.asarray(v) for k, v in inputs.items()})
    err = np.abs(got - expected)
    print("max_abs_err:", err.max(), " rel@scale:", err.max() / np.abs(expected).max())



# revision 10
# speedup vs baseline: 1.1826x; 1.1826x over previous
"""Trainium2 Bass kernel for nn_CorefModel (LSTM + span pooling + mention MLP +
windowed pairwise precedent MLP + softmax).

Sharding: data-parallel over batch B=8 across the 8 NeuronCores (one batch row
per core, all parameters replicated). No collectives.

Per-core pipeline (all layouts transposed so the partition dim is 128):
  A) indirect-DMA embedding gather -> fp16 -> DRAM -> transposing DMA -> we^T
  B) X^T = Wih^T @ we^T computed straight into PSUM (all 8 banks: bank j =
     gate-chunk j, col t = timestep). The LSTM bias is folded in via a
     ones-row appended to we^T and a bias row appended to Wih (so PSUM holds
     x_t W + b and the recurrence just accumulates on top).
  C) 512-step LSTM recurrence. Gate banks are laid out [g0 i0 f0 o0 g1 i1 f1
     o1] (halves of the 256-dim cell), so each half needs 1 tanh + 1 sigmoid
     ACT instruction reading PSUM columns directly. Per step: 16 accumulate
     matmuls (Whh fp16 stationary blocks x h fp16), 6 ACT, 8 DVE
     instructions; h is written once, as fp16, directly into the seq^T
     history (which doubles as the next step's matmul rhs). The two halves
     pipeline across engines: half-1 gates activate while half-0's cell
     updates run, and the next step's k=0 matmuls start as soon as h-half-0
     lands.
  D) span pooling: PE transpose seq^T -> seq, span sums as matmul against a
     host-built 0/1 indicator, PE transpose back -> tgt^T.
  E) mention MLP (fp32, transposed so biases are per-partition scalars).
  F) pairwise MLP (fp16): feat^T built with sliding-window / broadcast access
     patterns (precedent window j = i-50+k is just a shifted slice), 2-layer
     MLP in N=512 blocks, scalar head via K-partition-reduction matmuls.
  G) scores + masked softmax. softmax shift-invariance removes the ms_i
     broadcast: cols 0:50 = ms_j + ps + mask, epsilon col = -ms_i.
"""
import numpy as np

B, W, M, P = 8, 512, 128, 50
V, E, L, H = 50000, 300, 256, 512
G = 4 * L
NCORES = 8
NEG_INF = -1.0e30

_CACHE = {}


# ---------------------------------------------------------------- host prep --
def _perm_banks():
    """Device gate-bank order [g0 i0 f0 o0 g1 i1 f1 o1] (halves of L=256).
    Reference gate order is (i, f, g, o) in chunks of 256."""
    i0, f0, g0, o0 = np.arange(0, 128), np.arange(256, 384), np.arange(512, 640), np.arange(768, 896)
    return np.concatenate([g0, i0, f0, o0, g0 + 128, i0 + 128, f0 + 128, o0 + 128])


def _blocked(w, kchunks, hchunks):
    """[K,HH] -> [128, kchunks*hchunks*128] with col block (k*hchunks+h)*128."""
    K, HH = w.shape
    out = np.zeros((128, kchunks * hchunks * 128), w.dtype)
    for k in range(kchunks):
        kp = min(128, K - k * 128)
        for h in range(hchunks):
            blk = w[k * 128:k * 128 + kp, h * 128:(h + 1) * 128]
            out[:kp, (k * hchunks + h) * 128:(k * hchunks + h + 1) * 128] = blk
    return out


def _chunk_cols(v, n):
    """[n*128] -> [128, n] (col j = chunk j)."""
    return np.ascontiguousarray(v.reshape(n, 128).T)


def _prep_shared(inputs):
    f32, f16 = np.float32, np.float16
    perm = _perm_banks()
    Wih = np.asarray(inputs["Wih"], f32)[:, perm]
    Whh = np.asarray(inputs["Whh"], f32)[:, perm]
    bias = (np.asarray(inputs["bih"], f32) + np.asarray(inputs["bhh"], f32))[perm]

    # rows 0:300 = Wih, row 300 = bias (matched by a ones-row in we^T)
    wih_pad = np.zeros((304, G), f16)
    wih_pad[:E] = Wih.astype(f16)
    wih_pad[E] = bias.astype(f16)

    i_idx = np.arange(M)[:, None]
    k_idx = np.arange(P)[None, :]
    valid = k_idx < np.minimum(i_idx, P)
    maskinf = np.where(valid, 0.0, NEG_INF).astype(f32)

    return {
        "emb": np.asarray(inputs["emb"], f32),
        "wih16": wih_pad,
        "whh16": Whh.astype(f16),
        "wm1": _blocked(np.asarray(inputs["Wm1"], f32), 2, 4),
        "wm2": _blocked(np.asarray(inputs["Wm2"], f32), 4, 4),
        "bm": np.concatenate([_chunk_cols(np.asarray(inputs["bm1"], f32), 4),
                              _chunk_cols(np.asarray(inputs["bm2"], f32), 4)], 1),
        "wmv": _chunk_cols(np.asarray(inputs["wm"], f32), 4),
        "wa1": _blocked(np.asarray(inputs["Wa1"], np.float32).astype(f16), 6, 4),
        "wa2": _blocked(np.asarray(inputs["Wa2"], np.float32).astype(f16), 4, 4),
        "ba": np.concatenate([_chunk_cols(np.asarray(inputs["ba1"], f32), 4),
                              _chunk_cols(np.asarray(inputs["ba2"], f32), 4)], 1),
        "wav": _chunk_cols(np.asarray(inputs["wa"], np.float32), 4).astype(f16),
        "maskinf": maskinf,
        "ident16": np.eye(128, dtype=f16),
    }


def _prep_core(inputs, b):
    f32 = np.float32
    word = np.asarray(inputs["word_seq"][b], np.int32)
    starts = np.asarray(inputs["span_starts"][b], np.int64)
    lens = np.asarray(inputs["span_lengths"][b], np.int64)
    ends = np.clip(starts + lens, 0, W)
    t_idx = np.arange(W)[:, None]
    ind_full = ((t_idx >= starts[None, :]) & (t_idx < ends[None, :])).astype(f32)
    # ind[p, q*128+m] = ind_full[q*128+p, m]
    ind = np.ascontiguousarray(
        ind_full.reshape(4, 128, M).transpose(1, 0, 2).reshape(128, 4 * M)
    ).astype(np.float16)
    widx = np.ascontiguousarray(word.reshape(4, 128).T).astype(np.int32)
    return {"widx": widx, "ind": ind}


# ------------------------------------------------------------ program build --
def _build_program():
    import concourse.bacc as bacc
    import concourse.tile as tile
    from concourse import mybir
    import concourse.bass as bass

    f32, f16, i32 = mybir.dt.float32, mybir.dt.float16, mybir.dt.int32
    AF = mybir.ActivationFunctionType
    OP = mybir.AluOpType

    nc = bacc.Bacc("TRN2", target_bir_lowering=False, debug=False)

    def din(name, shape, dt):
        return nc.dram_tensor(name, shape, dt, kind="ExternalInput").ap()

    emb_d = din("emb", [V, E], f32)
    widx_d = din("widx", [128, 4], i32)
    wih_d = din("wih16", [304, G], f16)
    whh_d = din("whh16", [L, G], f16)
    ind_d = din("ind", [128, 4 * M], f16)
    wm1_d = din("wm1", [128, 2 * 4 * 128], f32)
    wm2_d = din("wm2", [128, 4 * 4 * 128], f32)
    bm_d = din("bm", [128, 8], f32)
    wmv_d = din("wmv", [128, 4], f32)
    wa1_d = din("wa1", [128, 6 * 4 * 128], f16)
    wa2_d = din("wa2", [128, 4 * 4 * 128], f16)
    ba_d = din("ba", [128, 8], f32)
    wav_d = din("wav", [128, 4], f16)
    mask_d = din("maskinf", [128, P], f32)
    ident_d = din("ident16", [128, 128], f16)

    we16_d = nc.dram_tensor("we16s", [W, 384], f16).ap()
    ms_d = nc.dram_tensor("mss", [M, 1], f32).ap()
    ps_d = nc.dram_tensor("pss", [1, M * P], f32).ap()
    out_d = nc.dram_tensor("o", [M, P + 1], f32, kind="ExternalOutput").ap()

    def ap3(base, off_elems, dims):
        """Manual AP on the same tensor: dims = [[stride, num], ...] (free),
        partition dim copied from base."""
        return bass.AP(tensor=base.tensor, offset=base.offset + off_elems,
                       ap=[base.ap[0]] + dims)

    with tile.TileContext(nc) as tc:
        from contextlib import ExitStack
        ctx = ExitStack()
        with ctx:
            singles = ctx.enter_context(tc.tile_pool(name="singles", bufs=1))
            bigctx = ExitStack()
            bigs = bigctx.enter_context(tc.tile_pool(name="bigs", bufs=1))

            # ---- LSTM-phase tensors (freed before pairwise phase) ------------
            weT = bigs.tile([128, 3, W], f16)       # we^T k-chunks (+ones row)
            wih_sb = bigs.tile([128, 3, 8, 128], f16)
            whh_sb = bigs.tile([128, 2, 8, 128], f16)
            seqT = bigs.tile([128, 2, W], f16)      # h^T history (fp16)
            ident_sb = bigs.tile([128, 128], f16)
            ind_sb = bigs.tile([128, 4, M], f16)
            c32 = bigs.tile([128, 2], f32)

            # ---- persistent SBUF tensors -------------------------------------
            wm1_sb = singles.tile([128, 2, 4, 128], f32)
            wm2_sb = singles.tile([128, 4, 4, 128], f32)
            bm_sb = singles.tile([128, 8], f32)
            wmv_sb = singles.tile([128, 4], f32)
            wa1_sb = singles.tile([128, 6, 4, 128], f16)
            wa2_sb = singles.tile([128, 4, 4, 128], f16)
            ba_sb = singles.tile([128, 8], f32)
            wav_sb = singles.tile([128, 4], f16)
            mask_sb = singles.tile([128, P], f32)
            tgtT32 = singles.tile([128, 2, M], f32)
            tgtT16 = singles.tile([128, 2, M], f16)
            m1T = singles.tile([128, 4, M], f32)
            m2T = singles.tile([128, 4, M], f32)
            ms_sb = singles.tile([1, M], f32)
            msi_sb = singles.tile([128, 1], f32)
            msj_sb = singles.tile([128, P], f32)
            psM_sb = singles.tile([128, P], f32)
            idx_sb = singles.tile([128, 4], i32)

            # weight / static DMAs (no deps -> scheduled early)
            nc.sync.dma_start(out=idx_sb[:], in_=widx_d[:])
            for k in range(3):
                kp = 128 if k < 2 else 48
                nc.sync.dma_start(out=wih_sb[0:kp, k, :, :],
                                  in_=wih_d[k * 128:k * 128 + kp, :])
            for k in range(2):
                nc.sync.dma_start(out=whh_sb[:, k, :, :],
                                  in_=whh_d[k * 128:(k + 1) * 128, :])
            nc.sync.dma_start(out=ident_sb[:], in_=ident_d[:])
            nc.sync.dma_start(out=ind_sb[:], in_=ind_d[:])
            nc.sync.dma_start(out=wm1_sb[:], in_=wm1_d[:])
            nc.sync.dma_start(out=wm2_sb[:], in_=wm2_d[:])
            nc.sync.dma_start(out=bm_sb[:], in_=bm_d[:])
            nc.sync.dma_start(out=wmv_sb[:], in_=wmv_d[:])
            nc.sync.dma_start(out=wa1_sb[:], in_=wa1_d[:])
            nc.sync.dma_start(out=wa2_sb[:], in_=wa2_d[:])
            nc.sync.dma_start(out=ba_sb[:], in_=ba_d[:])
            nc.sync.dma_start(out=wav_sb[:], in_=wav_d[:])
            nc.sync.dma_start(out=mask_sb[:], in_=mask_d[:])

            # ---- phase A: embedding gather + transpose -----------------------
            with tc.tile_pool(name="gath", bufs=2) as gpool:
                for g in range(4):
                    wet = gpool.tile([128, 384], f32, tag="wet")
                    # col 300 = ones (matches the bias row of wih); rest pad 0
                    nc.vector.memset(wet[:, E:E + 1], 1.0)
                    nc.vector.memset(wet[:, E + 1:384], 0.0)
                    nc.gpsimd.indirect_dma_start(
                        out=wet[:, 0:E], out_offset=None, in_=emb_d[:],
                        in_offset=bass.IndirectOffsetOnAxis(
                            ap=idx_sb[:, g:g + 1], axis=0))
                    # cast f32 -> f16 during DMA (SWDGE)
                    nc.gpsimd.dma_start(out=we16_d[g * 128:(g + 1) * 128, :],
                                        in_=wet[:])
                for c in range(3):
                    nc.sync.dma_start(out=weT[:, c, :],
                                      in_=we16_d[:, c * 128:(c + 1) * 128], transpose=True)

            # ---- phases B+C share the full PSUM (bank j = gate chunk j) ------
            with tc.tile_pool(name="xps", bufs=1, space="PSUM") as xpool, \
                 tc.tile_pool(name="lsb", bufs=3) as lsb:
                xps = xpool.tile([128, 8, W], f32)

                # phase B: X^T + bias -> PSUM banks
                for j in range(8):
                    for k, kp in enumerate([128, 128, 45]):
                        nc.tensor.matmul(out=xps[:, j, :],
                                         lhsT=wih_sb[0:kp, k, j, :],
                                         rhs=weT[0:kp, k, :],
                                         start=(k == 0), stop=(k == 2))

                # phase C: LSTM recurrence
                nc.vector.memset(c32[:], 0.0)
                for t in range(W):
                    if t > 0:
                        # accumulate Whh @ h_{t-1} onto X in PSUM.
                        # half-0 output banks first; within each, k=0 (h half
                        # 0) before k=1 so the PE can start on h0 early.
                        for half in range(2):
                            for k in range(2):
                                for j in range(4 * half, 4 * half + 4):
                                    nc.tensor.matmul(
                                        out=xps[:, j, t:t + 1],
                                        lhsT=whh_sb[:, k, j, :],
                                        rhs=seqT[:, k, t - 1:t],
                                        start=False, stop=(k == 1),
                                        skip_group_check=True)
                    ga = lsb.tile([128, 8], f32, tag="ga")
                    # gates: per half, tanh(g) + sigmoid(i,f,o) from PSUM
                    for half in range(2):
                        nc.scalar.activation(out=ga[:, 4 * half:4 * half + 1],
                                             in_=xps[:, 4 * half, t:t + 1],
                                             func=AF.Tanh)
                        nc.scalar.activation(out=ga[:, 4 * half + 1:4 * half + 4],
                                             in_=xps[:, 4 * half + 1:4 * half + 4, t],
                                             func=AF.Sigmoid)
                    tc_t = lsb.tile([128, 2], f32, tag="tc")
                    if t == 0:
                        for half in range(2):
                            nc.vector.tensor_tensor(
                                out=c32[:, half:half + 1],
                                in0=ga[:, 4 * half:4 * half + 1],
                                in1=ga[:, 4 * half + 1:4 * half + 2], op=OP.mult)
                            nc.scalar.activation(out=tc_t[:, half:half + 1],
                                                 in_=c32[:, half:half + 1],
                                                 func=AF.Tanh)
                    else:
                        igfc = lsb.tile([128, 4], f32, tag="igfc")
                        for half in range(2):
                            # ig = tanh(g) * sig(i); fc = sig(f) * c
                            nc.vector.tensor_tensor(
                                out=igfc[:, 2 * half:2 * half + 1],
                                in0=ga[:, 4 * half:4 * half + 1],
                                in1=ga[:, 4 * half + 1:4 * half + 2], op=OP.mult)
                            nc.vector.tensor_tensor(
                                out=igfc[:, 2 * half + 1:2 * half + 2],
                                in0=ga[:, 4 * half + 2:4 * half + 3],
                                in1=c32[:, half:half + 1], op=OP.mult)
                            nc.vector.tensor_tensor(
                                out=c32[:, half:half + 1],
                                in0=igfc[:, 2 * half:2 * half + 1],
                                in1=igfc[:, 2 * half + 1:2 * half + 2], op=OP.add)
                            nc.scalar.activation(out=tc_t[:, half:half + 1],
                                                 in_=c32[:, half:half + 1],
                                                 func=AF.Tanh)
                    # h = sig(o) * tanh(c), straight into the fp16 history
                    for half in range(2):
                        nc.vector.tensor_tensor(
                            out=seqT[:, half, t:t + 1],
                            in0=ga[:, 4 * half + 3:4 * half + 4],
                            in1=tc_t[:, half:half + 1], op=OP.mult)

            # ---- phase D: span pooling ---------------------------------------
            with tc.tile_pool(name="dps", bufs=4, space="PSUM") as dps, \
                 tc.tile_pool(name="dsb", bufs=2) as dsb:
                tgt_ps = dps.tile([128, 2 * 128], f32, tag="tgt")
                for q in range(4):
                    seq_q = dsb.tile([128, 2, 128], f16, tag="seqq")
                    for c in range(2):
                        pt = dps.tile([128, 128], f16, tag="pt")
                        nc.tensor.transpose(out=pt[:],
                                            in_=seqT[:, c, q * 128:(q + 1) * 128],
                                            identity=ident_sb[:])
                        nc.vector.tensor_copy(out=seq_q[:, c, :], in_=pt[:])
                    nc.tensor.matmul(out=tgt_ps[:], lhsT=ind_sb[:, q, :],
                                     rhs=seq_q[:].rearrange("p c t -> p (c t)"),
                                     start=(q == 0), stop=(q == 3))
                tgt_sb = dsb.tile([128, 256], f16, tag="tgtsb")
                nc.vector.tensor_copy(out=tgt_sb[:], in_=tgt_ps[:])
                for c in range(2):
                    pt2 = dps.tile([128, 128], f16, tag="pt")
                    nc.tensor.transpose(out=pt2[:],
                                        in_=tgt_sb[:, c * 128:(c + 1) * 128],
                                        identity=ident_sb[:])
                    nc.vector.tensor_copy(out=tgtT32[:, c, :], in_=pt2[:])
                    nc.vector.tensor_copy(out=tgtT16[:, c, :], in_=pt2[:])

            bigctx.close()  # free LSTM-phase SBUF before pairwise

            # ---- phase E: mention MLP + ms -----------------------------------
            with tc.tile_pool(name="eps", bufs=2, space="PSUM") as eps:
                for h in range(4):
                    pm = eps.tile([128, M], f32, tag="pm")
                    for k in range(2):
                        nc.tensor.matmul(out=pm[:], lhsT=wm1_sb[:, k, h, :],
                                         rhs=tgtT32[:, k, :],
                                         start=(k == 0), stop=(k == 1))
                    nc.scalar.activation(out=m1T[:, h, :], in_=pm[:],
                                         func=AF.Relu, bias=bm_sb[:, h:h + 1])
                for h in range(4):
                    pm = eps.tile([128, M], f32, tag="pm")
                    for k in range(4):
                        nc.tensor.matmul(out=pm[:], lhsT=wm2_sb[:, k, h, :],
                                         rhs=m1T[:, k, :],
                                         start=(k == 0), stop=(k == 3))
                    nc.scalar.activation(out=m2T[:, h, :], in_=pm[:],
                                         func=AF.Relu, bias=bm_sb[:, 4 + h:5 + h])
                pms = eps.tile([1, M], f32, tag="pms")
                for k in range(4):
                    nc.tensor.matmul(out=pms[:], lhsT=wmv_sb[:, k:k + 1],
                                     rhs=m2T[:, k, :],
                                     start=(k == 0), stop=(k == 3))
                nc.vector.tensor_copy(out=ms_sb[:], in_=pms[:])
                nc.sync.dma_start(out=ms_d[:], in_=ms_sb[:])
                # ms_i per-partition
                nc.sync.dma_start(out=msi_sb[:], in_=ms_d[:])
                # ms_j sliding window: i>=50 -> ms[i-50+k]; i<50 -> ms[k]
                nc.sync.dma_start(
                    out=msj_sb[P:M, :],
                    in_=bass.AP(tensor=ms_d.tensor, offset=0,
                                ap=[[1, M - P], [1, P]]))
                nc.sync.dma_start(
                    out=msj_sb[0:P, :],
                    in_=bass.AP(tensor=ms_d.tensor, offset=0,
                                ap=[[0, P], [1, P]]))

            # ---- phase F: pairwise MLP ---------------------------------------
            NPAIR = M * P  # 6400
            NA = P * P     # 2500 (region i<50)
            blocks = [(s, min(512, NPAIR - s)) for s in range(0, NPAIR, 512)]
            with tc.tile_pool(name="h1p", bufs=1) as h1p, \
                 tc.tile_pool(name="fps", bufs=2, space="PSUM") as fps, \
                 tc.tile_pool(name="fpssb", bufs=3) as fps_sb:
                h1T = h1p.tile([128, 4, NPAIR], f16)
                featp = ExitStack()
                featpool = featp.enter_context(tc.tile_pool(name="feat", bufs=1))
                featT = featpool.tile([128, 6, NPAIR], f16)
                for c in range(2):
                    base = tgtT16[:, c, :]       # [128, 128] fp16
                    # jvec (chunk c): A: tgt[k]; B: tgt[i-50+k]
                    nc.vector.tensor_copy(
                        out=featT[:, c, 0:NA].rearrange("p (i k) -> p i k", k=P),
                        in_=ap3(base, 0, [[0, P], [1, P]]))
                    nc.vector.tensor_copy(
                        out=featT[:, c, NA:NPAIR].rearrange("p (i k) -> p i k", k=P),
                        in_=ap3(base, 0, [[1, M - P], [1, P]]))
                    # ivec (chunk 2+c): A: tgt[i]; B: tgt[i]
                    nc.vector.tensor_copy(
                        out=featT[:, 2 + c, 0:NA].rearrange("p (i k) -> p i k", k=P),
                        in_=ap3(base, 0, [[1, P], [0, P]]))
                    nc.vector.tensor_copy(
                        out=featT[:, 2 + c, NA:NPAIR].rearrange("p (i k) -> p i k", k=P),
                        in_=ap3(base, P, [[1, M - P], [0, P]]))
                    # prod (chunk 4+c)
                    nc.vector.tensor_tensor(out=featT[:, 4 + c, :],
                                            in0=featT[:, c, :],
                                            in1=featT[:, 2 + c, :], op=OP.mult)
                for n0, nb in blocks:
                    for h in range(4):
                        p1 = fps.tile([128, 512], f32, tag="p1")
                        for k in range(6):
                            nc.tensor.matmul(out=p1[:, 0:nb],
                                             lhsT=wa1_sb[:, k, h, :],
                                             rhs=featT[:, k, n0:n0 + nb],
                                             start=(k == 0), stop=(k == 5))
                        nc.scalar.activation(out=h1T[:, h, n0:n0 + nb],
                                             in_=p1[:, 0:nb], func=AF.Relu,
                                             bias=ba_sb[:, h:h + 1])
                featp.close()  # free featT before h2T allocates
                with tc.tile_pool(name="h2p", bufs=1) as h2p:
                    h2T = h2p.tile([128, 4, NPAIR], f16)
                    for n0, nb in blocks:
                        for h in range(4):
                            p2 = fps.tile([128, 512], f32, tag="p1")
                            for k in range(4):
                                nc.tensor.matmul(out=p2[:, 0:nb],
                                                 lhsT=wa2_sb[:, k, h, :],
                                                 rhs=h1T[:, k, n0:n0 + nb],
                                                 start=(k == 0), stop=(k == 3))
                            nc.scalar.activation(out=h2T[:, h, n0:n0 + nb],
                                                 in_=p2[:, 0:nb], func=AF.Relu,
                                                 bias=ba_sb[:, 4 + h:5 + h])
                    for n0, nb in blocks:
                        pps = fps.tile([1, 512], f32, tag="pps")
                        for k in range(4):
                            nc.tensor.matmul(out=pps[:, 0:nb],
                                             lhsT=wav_sb[:, k:k + 1],
                                             rhs=h2T[:, k, n0:n0 + nb],
                                             start=(k == 0), stop=(k == 3))
                        pse = fps_sb.tile([1, 512], f32, tag="pse",
                                          name=f"pse_{n0}")
                        nc.vector.tensor_copy(out=pse[:, 0:nb],
                                              in_=pps[:, 0:nb])
                        nc.sync.dma_start(out=ps_d[:, n0:n0 + nb],
                                          in_=pse[:, 0:nb])
                nc.sync.dma_start(
                    out=psM_sb[:],
                    in_=bass.AP(tensor=ps_d.tensor, offset=0,
                                ap=[[P, M], [1, P]]))

            # ---- phase G: scores + softmax -----------------------------------
            with tc.tile_pool(name="gsb", bufs=1) as gsb:
                sc = gsb.tile([128, P + 1], f32)
                nc.vector.tensor_tensor(out=sc[:, 0:P], in0=psM_sb[:],
                                        in1=msj_sb[:], op=OP.add)
                nc.vector.tensor_tensor(out=sc[:, 0:P], in0=sc[:, 0:P],
                                        in1=mask_sb[:], op=OP.add)
                nc.vector.tensor_scalar_mul(sc[:, P:P + 1], msi_sb[:], -1.0)
                mx = gsb.tile([128, 1], f32)
                nc.vector.tensor_reduce(out=mx[:], in_=sc[:],
                                        axis=mybir.AxisListType.X,
                                        op=OP.max, negate=True)
                ex = gsb.tile([128, P + 1], f32)
                sm = gsb.tile([128, 1], f32)
                nc.scalar.activation(out=ex[:], in_=sc[:], func=AF.Exp,
                                     bias=mx[:], accum_out=sm[:])
                rs = gsb.tile([128, 1], f32)
                nc.vector.reciprocal(out=rs[:], in_=sm[:])
                ot = gsb.tile([128, P + 1], f32)
                nc.vector.tensor_scalar_mul(ot[:], ex[:], rs[:])
                nc.sync.dma_start(out=out_d[:], in_=ot[:])

    nc.compile()
    return nc


# -------------------------------------------------------------------- entry --
def kernel(**inputs):
    import os
    from concourse.bass_utils import run_bass_kernel_spmd

    if "nc" not in _CACHE:
        _CACHE["nc"] = _build_program()
    nc = _CACHE["nc"]

    shared = _prep_shared(inputs)
    in_maps = []
    for b in range(NCORES):
        m = dict(shared)
        m.update(_prep_core(inputs, b))
        in_maps.append(m)

    trace = bool(os.environ.get("COREF_TRACE"))
    res = run_bass_kernel_spmd(nc, in_maps, core_ids=list(range(NCORES)),
                               trace=trace)
    kernel.last_exec_ns = res.exec_time_ns
    kernel.last_results = res
    out = np.stack([res.results[i]["o"] for i in range(NCORES)])
    return out.astype(np.float32)


if __name__ == "__main__":
    import jax
    jax.config.update("jax_platforms", "cpu")
    import reference as ref
    inputs = ref.setup_inputs()
    expected = np.asarray(jax.device_get(ref.reference(**inputs)))
    got = kernel(**{k: np.asarray(v) for k, v in inputs.items()})
    err = np.abs(got - expected)
    print("max_abs_err:", err.max(), " rel@scale:", err.max() / np.abs(expected).max())
